# revision 10
# baseline (speedup 1.0000x reference)
"""BitfieldLinear (vq_codebook) Trainium2 kernel — fp8 decomposed.

y = x @ W^T + bias, W = r[:,None]*basis[idx] + s[:,None]*(q-128)/127.

Instead of materializing W in bf16 (PE floor ~437us/core), split:
  y = zT.T @ G + x_f8 @ residT_f8 * (1/C) + bias
  - z = x @ basisT in bf16 ([tokens, 256 basis]); the basis term
    becomes a one-hot gather matmul with G[b,o] = C*r[o] at b=idx[o]
    (accurate path, 1/8 the flops).
  - the residual matmul runs in fp8e4 DoubleRow perf mode (2x PE rate);
    residT_f8 pre-scaled by s*C/127 (C=2048 keeps values out of the
    fp8 subnormal range), x cast bf16->fp8 directly.
  Both accumulate into the same PSUM bank; evacuation is one DVE op
  (psum * 1/C + bias_bc).

Sharding across 8 NeuronCores: 2-way over out_features x 4-way over
flattened tokens. Per core: 2048 tokens x 2048 outs, K=4096.
  - resid rows stream: int32 DMA -> ACT/DVE decode (s*C/127 folded,
    bf16) -> xbar transpose -> DVE cast to residT_f8 [128,32k,2048o].
  - x streams via gpsimd cast-DMA (f32->bf16) -> xbar -> fp8 cast;
    z chains + PE transposes of z run during the resid build window.
Host only slices inputs and reassembles the output.
"""

import numpy as np

import concourse.bass as bass
import concourse.mybir as mybir
import concourse.tile as tile
from concourse.masks import make_identity
from concourse.bass_utils import run_bass_kernel_spmd

# problem shape (hardcoded per harness contract)
B, S, D_IN, D_OUT, BASIS = 4, 2048, 4096, 4096, 256
N_CORES = 8
O_SHARDS, N_SHARDS = 2, 4           # grid: core = oc * N_SHARDS + nb
O_SH = D_OUT // O_SHARDS            # 2048 out-features per core
N_SH = (B * S) // N_SHARDS          # 2048 token rows per core

P = 128
KC = D_IN // P                      # 32 contraction chunks
KH = KC // 2                        # 16 chunks per D_IN half
HALF = D_IN // 2
NB = N_SH // P                      # 16 token blocks per core
NOS = O_SH // 512                   # 4 PSUM o-slices per core
OT = O_SH // P                      # 16 resid row-tiles per core
CSC = 2048.0                        # fp8 residual pre-scale

F32 = mybir.dt.float32
BF16 = mybir.dt.bfloat16
F8 = mybir.dt.float8e4
I32 = mybir.dt.int32

_WAIT_LIMIT = 1


def _split_sync_waits(nc):
    """walrus in this container rejects instructions with more than one
    embedded sync-wait command; hoist the excess onto same-engine NoOps."""
    ctr = 0
    for f in nc.m.functions:
        for bb in f.blocks:
            new = []
            changed = False
            for inst in bb.instructions:
                si = inst.sync_info
                if si is not None and si.on_wait and len(si.on_wait) > _WAIT_LIMIT:
                    waits = list(si.on_wait)
                    excess, keep = waits[:-_WAIT_LIMIT], waits[-_WAIT_LIMIT:]
                    for i in range(0, len(excess), _WAIT_LIMIT):
                        ctr += 1
                        new.append(mybir.InstNoOp(
                            name=f"I-waitsplit-{ctr}",
                            engine=inst.engine,
                            ins=[], outs=[],
                            sync_info=mybir.SyncInfo(
                                on_wait=excess[i:i + _WAIT_LIMIT], on_update=[]),
                        ))
                    si.on_wait = keep
                    changed = True
                new.append(inst)
            if changed:
                bb.instructions = new


def _build_program(split_waits=True):
    nc = bass.Bass()
    Alu = mybir.AluOpType
    Act = mybir.ActivationFunctionType
    DR = mybir.MatmulPerfMode.DoubleRow

    x_in = nc.dram_tensor("x_sh", [N_SH, D_IN], F32, kind="ExternalInput")
    codes_in = nc.dram_tensor("codes_sh", [O_SH], I32, kind="ExternalInput")
    basis_in = nc.dram_tensor("basis", [BASIS, D_IN], F32, kind="ExternalInput")
    resid_in = nc.dram_tensor("resid_sh", [O_SH, D_IN], I32, kind="ExternalInput")
    scales_in = nc.dram_tensor("scales_sh", [O_SH], F32, kind="ExternalInput")
    bias_in = nc.dram_tensor("bias_sh", [O_SH], F32, kind="ExternalInput")
    y_out = nc.dram_tensor("y_sh", [N_SH, O_SH], F32, kind="ExternalOutput")

    with tile.TileContext(nc) as tc:
        with (
            tc.tile_pool(name="const", bufs=1) as cpool,
            tc.tile_pool(name="rows", bufs=2) as rowpool,   # i32 scratch rows
            tc.tile_pool(name="xbf", bufs=3) as xbfpool,    # [128,2048] bf16
            tc.tile_pool(name="xt", bufs=4) as xtpool,      # [128,KH,128] bf16
            tc.tile_pool(name="xf8", bufs=5) as xf8pool,    # [128,KC,128] f8
            tc.tile_pool(name="zsb", bufs=2) as zsbpool,    # [128,256] bf16
            tc.tile_pool(name="r32", bufs=2) as r32pool,    # [128,HALF] i32
            tc.tile_pool(name="rbf", bufs=2) as rbfpool,    # [128,HALF] bf16
            tc.tile_pool(name="rt", bufs=2) as rtpool,      # [128,KH,P] bf16
            tc.tile_pool(name="y", bufs=2) as ypool,        # [128,512] f32
            tc.tile_pool(name="psum", bufs=6, space="PSUM") as pspool,
        ):
            # ---- decode code scalars --------------------------------
            codes_row = rowpool.tile([1, O_SH], I32, tag="row",
                                     name="codes_row")
            nc.sync.dma_start(codes_row[:], codes_in[None, :])
            tmp_row = rowpool.tile([1, O_SH], I32, tag="row", name="tmp_row")
            nc.vector.tensor_scalar(tmp_row[:], codes_row[:], 0xFF, None,
                                    Alu.bitwise_and)
            idx_row_f = cpool.tile([1, O_SH], BF16, name="idx_row_f")
            nc.scalar.activation(idx_row_f[:], tmp_row[:], Act.Copy)
            rq_row = rowpool.tile([1, O_SH], I32, tag="row", name="rq_row")
            nc.vector.tensor_scalar(rq_row[:], codes_row[:], 8, 0xFFFF,
                                    Alu.logical_shift_right, Alu.bitwise_and)
            # r scaled by C so the fp8 residual path's 1/C evac matches
            r_row_f = cpool.tile([1, O_SH], BF16, name="r_row_f")
            nc.scalar.activation(r_row_f[:], rq_row[:], Act.Copy,
                                 scale=CSC / 65535.0)

            # per-row decode scale/bias for the residual (ACT layout)
            s_pp = cpool.tile([P, OT], F32, name="s_pp")
            nc.sync.dma_start(s_pp[:], scales_in.rearrange("(t p) -> p t", p=P))
            sv_pp = cpool.tile([P, OT], F32, name="sv_pp")
            nc.vector.tensor_scalar_mul(sv_pp[:], s_pp[:], CSC / 127.0)
            bv_pp = cpool.tile([P, OT], F32, name="bv_pp")
            nc.vector.tensor_scalar_mul(bv_pp[:], s_pp[:], -128.0 * CSC / 127.0)

            bias_row = cpool.tile([1, O_SH], BF16, name="bias_row")
            nc.gpsimd.dma_start(bias_row[:], bias_in[None, :])
            ones_row = cpool.tile([1, P], BF16, name="ones_row")
            nc.vector.memset(ones_row[:], 1.0)
            identity = cpool.tile([P, P], BF16, name="identity")
            make_identity(nc, identity[:])

            # ---- one-hot G [128 b, 2 bh, O_SH o] and bias broadcast --
            iota_i = cpool.tile([P, 1], I32, name="iota_i")
            nc.gpsimd.iota(iota_i[:], pattern=[[0, 1]], base=0,
                           channel_multiplier=1)
            iota_f = [cpool.tile([P, 1], F32, name=f"iota_f{bh}")
                      for bh in range(2)]
            nc.scalar.activation(iota_f[0][:], iota_i[:], Act.Copy)
            nc.scalar.activation(iota_f[1][:], iota_i[:], Act.Copy, bias=128.0,
                                 scale=1.0)
            G = cpool.tile([P, 2, O_SH], BF16, name="G")
            bias_bc = cpool.tile([P, O_SH], BF16, name="bias_bc")
            r_bc = cpool.tile([P, 512], BF16, name="r_bc")
            for q in range(NOS):
                qs = slice(q * 512, (q + 1) * 512)
                pr = pspool.tile([P, 512], F32, tag="bc", bufs=1, name=f"pr{q}")
                nc.tensor.matmul(pr[:], lhsT=ones_row[:], rhs=r_row_f[:, qs],
                                 start=True, stop=True)
                nc.scalar.copy(r_bc[:], pr[:])
                pi = pspool.tile([P, 512], F32, tag="bc", bufs=1, name=f"pi{q}")
                nc.tensor.matmul(pi[:], lhsT=ones_row[:], rhs=idx_row_f[:, qs],
                                 start=True, stop=True)
                for bh in range(2):
                    nc.vector.scalar_tensor_tensor(
                        G[:, bh, qs], pi[:], iota_f[bh][:, :1], r_bc[:],
                        op0=Alu.is_equal, op1=Alu.mult)
                pb = pspool.tile([P, 512], F32, tag="bc", bufs=1, name=f"pb{q}")
                nc.tensor.matmul(pb[:], lhsT=ones_row[:], rhs=bias_row[:, qs],
                                 start=True, stop=True)
                nc.scalar.copy(bias_bc[:, qs], pb[:])

            # ---- basisT [128 i, KC k, 256 b] bf16 --------------------
            basisT = cpool.tile([P, KC, BASIS], BF16, name="basisT")
            for bh2 in range(2):
                for hf in range(2):
                    hs = slice(hf * HALF, (hf + 1) * HALF)
                    b_bf = xbfpool.tile([P, HALF], BF16, tag="xbf",
                                        name=f"bbf{bh2}_{hf}")
                    nc.gpsimd.dma_start(b_bf[:],
                                        basis_in[bh2 * P:(bh2 + 1) * P, hs])
                    nc.sync.dma_start_transpose(
                        basisT[:, hf * KH:(hf + 1) * KH, bh2 * P:(bh2 + 1) * P],
                        b_bf[:])

            # ---- persistent W^T (residual, fp8) and zT ---------------
            residT = cpool.tile([P, KC, O_SH], F8, name="residT")
            zT = cpool.tile([P, 2, N_SH], BF16, name="zT")

            def resid_step(t, hf):
                # [128 o-rows, 2048 i-half]: load, decode, transpose, cast
                hs = slice(hf * HALF, (hf + 1) * HALF)
                eng_a = nc.scalar if (2 * t + hf) % 2 == 0 else nc.gpsimd
                rbf = rbfpool.tile([P, HALF], BF16, tag="rbf",
                                   name=f"rbf{t}_{hf}")
                for qh in range(2):
                    qsl = slice(qh * 1024, (qh + 1) * 1024)
                    qsrc = slice(hf * HALF + qh * 1024,
                                 hf * HALF + (qh + 1) * 1024)
                    r32 = r32pool.tile([P, 1024], I32, tag="r32",
                                       name=f"r32_{t}_{hf}_{qh}")
                    eng_a.dma_start(r32[:], resid_in[t * P:(t + 1) * P, qsrc])
                    if (2 * t + hf) % 2 == 0:
                        nc.scalar.activation(rbf[:, qsl], r32[:], Act.Identity,
                                             bias=bv_pp[:, t:t + 1],
                                             scale=sv_pp[:, t:t + 1])
                    else:
                        nc.vector.tensor_scalar(rbf[:, qsl], r32[:],
                                                sv_pp[:, t:t + 1],
                                                bv_pp[:, t:t + 1],
                                                Alu.mult, Alu.add)
                rt = rtpool.tile([P, KH, P], BF16, tag="rt", name=f"rt{t}_{hf}")
                nc.sync.dma_start_transpose(rt[:], rbf[:])
                nc.vector.tensor_copy(
                    residT[:, hf * KH:(hf + 1) * KH, t * P:(t + 1) * P], rt[:])

            def xz_block(tb):
                # one token block: x -> xT (bf16+f8), z chain, zT via PE
                # each xbar writes its own full tile (concurrent sliced
                # transpose-writes to one tile race on hardware)
                xth = []
                for hf in range(2):
                    hs = slice(hf * HALF, (hf + 1) * HALF)
                    xbf = xbfpool.tile([P, HALF], BF16, tag="xbf",
                                       name=f"xbf{tb}_{hf}")
                    nc.gpsimd.dma_start(xbf[:], x_in[tb * P:(tb + 1) * P, hs])
                    xt = xtpool.tile([P, KH, P], BF16, tag="xt",
                                     name=f"xt{tb}_{hf}")
                    nc.sync.dma_start_transpose(xt[:], xbf[:])
                    xth.append(xt)
                xf8 = xf8pool.tile([P, KC, P], F8, tag="xf8", name=f"xf8_{tb}")
                for hf in range(2):
                    nc.vector.tensor_copy(
                        xf8[:, hf * KH:(hf + 1) * KH, :], xth[hf][:])
                # z[tok, b] with full-width rhs, then PE-transpose to zT
                pz = pspool.tile([P, 512], F32, tag="z", bufs=2,
                                 name=f"pz{tb}")
                for k in range(KC):
                    nc.tensor.matmul(pz[:, :2 * P], lhsT=xth[k // KH][:, k % KH, :],
                                     rhs=basisT[:, k, :],
                                     start=(k == 0), stop=(k == KC - 1))
                z_sb = zsbpool.tile([P, 2 * P], BF16, tag="z", name=f"zsb{tb}")
                nc.scalar.copy(z_sb[:], pz[:, :2 * P])
                for bh in range(2):
                    # full-bank PSUM tile: sub-bank tiles corrupt each
                    # other through the bank-granular start=True zeroing
                    pzT = pspool.tile([P, 1024], BF16, tag="zt", bufs=2,
                                      name=f"pzT{tb}_{bh}")
                    nc.tensor.transpose(pzT[:, :P], z_sb[:, bh * P:(bh + 1) * P],
                                        identity[:])
                    nc.scalar.copy(zT[:, bh, tb * P:(tb + 1) * P], pzT[:, :P])
                return xf8

            xf8s = [None] * NB

            def main_mm(tb, os):
                osl = slice(os * 512, (os + 1) * 512)
                tbs = slice(tb * P, (tb + 1) * P)
                xf8 = xf8s[tb]
                ps = pspool.tile([P, 512], F32, tag="mm", bufs=3,
                                 name=f"ps{tb}_{os}")
                for bh in range(2):
                    nc.tensor.matmul(ps[:], lhsT=zT[:, bh, tbs],
                                     rhs=G[:, bh, osl],
                                     start=(bh == 0), stop=False)
                for k2 in range(KH):
                    ks = slice(2 * k2, 2 * k2 + 2)
                    nc.tensor.matmul(ps[:], lhsT=xf8[:, ks, :],
                                     rhs=residT[:, ks, osl],
                                     start=False, stop=(k2 == KH - 1),
                                     perf_mode=DR)
                y_t = ypool.tile([P, 512], F32, tag="y", name=f"y{tb}_{os}")
                nc.vector.scalar_tensor_tensor(y_t[:], ps[:], 1.0 / CSC,
                                               bias_bc[:, osl],
                                               op0=Alu.mult, op1=Alu.add)
                nc.scalar.dma_start(y_out[tbs, osl], y_t[:])

            # ---- interleaved schedule (issue order = engine order) ---
            for t in (0, 1, 2, 3):
                resid_step(t, 0), resid_step(t, 1)
            xf8s[0] = xz_block(0)
            xf8s[1] = xz_block(1)
            for t in (4, 5):
                resid_step(t, 0), resid_step(t, 1)
            xf8s[2] = xz_block(2)
            xf8s[3] = xz_block(3)
            for t in (6, 7):
                resid_step(t, 0), resid_step(t, 1)
            main_mm(0, 0), main_mm(1, 0)
            for t in (8, 9):
                resid_step(t, 0), resid_step(t, 1)
            main_mm(2, 0), main_mm(3, 0)
            for t in (10, 11):
                resid_step(t, 0), resid_step(t, 1)
            main_mm(0, 1), main_mm(1, 1)
            for t in (12, 13):
                resid_step(t, 0), resid_step(t, 1)
            main_mm(2, 1), main_mm(3, 1)
            for t in (14, 15):
                resid_step(t, 0), resid_step(t, 1)
            for tb in range(4):
                main_mm(tb, 2)
            for tb in range(4):
                main_mm(tb, 3)
            xf8s[4] = xz_block(4)
            for tb in range(4, NB):
                if tb + 1 < NB:
                    xf8s[tb + 1] = xz_block(tb + 1)
                for os in range(NOS):
                    main_mm(tb, os)

    if split_waits:
        _split_sync_waits(nc)
    return nc


_program_cache = {}


def _get_program():
    if "nc" not in _program_cache:
        _program_cache["nc"] = _build_program()
    return _program_cache["nc"]


def _in_maps(x, codes, basis_table, residual_q, residual_scales, bias):
    x2 = x.reshape(B * S, D_IN)
    in_maps = []
    for core in range(N_CORES):
        oc, nb = divmod(core, N_SHARDS)
        osl = slice(oc * O_SH, (oc + 1) * O_SH)
        nsl = slice(nb * N_SH, (nb + 1) * N_SH)
        in_maps.append({
            "x_sh": np.ascontiguousarray(x2[nsl]),
            "codes_sh": np.ascontiguousarray(codes[osl]),
            "basis": basis_table,
            "resid_sh": np.ascontiguousarray(residual_q[osl]),
            "scales_sh": np.ascontiguousarray(residual_scales[osl]),
            "bias_sh": np.ascontiguousarray(bias[osl]),
        })
    return in_maps


def kernel(x, codes, basis_table, residual_q, residual_scales, bias):
    x = np.ascontiguousarray(np.asarray(x, dtype=np.float32))
    codes = np.ascontiguousarray(np.asarray(codes, dtype=np.int32))
    basis_table = np.ascontiguousarray(np.asarray(basis_table, dtype=np.float32))
    residual_q = np.ascontiguousarray(np.asarray(residual_q, dtype=np.int32))
    residual_scales = np.ascontiguousarray(
        np.asarray(residual_scales, dtype=np.float32))
    bias = np.ascontiguousarray(np.asarray(bias, dtype=np.float32))

    nc = _get_program()
    res = run_bass_kernel_spmd(nc, _in_maps(x, codes, basis_table, residual_q,
                                            residual_scales, bias),
                               core_ids=list(range(N_CORES)))

    y = np.empty((B * S, D_OUT), dtype=np.float32)
    for core in range(N_CORES):
        oc, nb = divmod(core, N_SHARDS)
        y[nb * N_SH:(nb + 1) * N_SH, oc * O_SH:(oc + 1) * O_SH] = \
            res.results[core]["y_sh"]
    return y.reshape(B, S, D_OUT)


# revision 12
# speedup vs baseline: 1.2068x; 1.2068x over previous
"""BitfieldLinear (vq_codebook) Trainium2 kernel — fp8 decomposed.

y = x @ W^T + bias, W = r[:,None]*basis[idx] + s[:,None]*(q-128)/127.

Instead of materializing W in bf16 (PE floor ~437us/core), split:
  y = zT.T @ G + x_f8 @ residT_f8 * (1/C) + bias
  - z = x @ basisT in bf16 ([tokens, 256 basis]); the basis term
    becomes a one-hot gather matmul with G[b,o] = C*r[o] at b=idx[o]
    (accurate path, 1/8 the flops).
  - the residual matmul runs in fp8e4 DoubleRow perf mode (2x PE rate);
    residT_f8 pre-scaled by s*C/127 (C=2048 keeps values out of the
    fp8 subnormal range), x cast bf16->fp8 directly.
  Both accumulate into the same PSUM bank; evacuation is one DVE op
  (psum * 1/C + bias_bc).

Sharding across 8 NeuronCores: 2-way over out_features x 4-way over
flattened tokens. Per core: 2048 tokens x 2048 outs, K=4096.
  - resid rows stream: int32 DMA -> ACT/DVE decode (s*C/127 folded,
    bf16) -> xbar transpose -> DVE cast to residT_f8 [128,32k,2048o].
  - x streams via gpsimd cast-DMA (f32->bf16) -> xbar -> fp8 cast;
    z chains + PE transposes of z run during the resid build window.
Host only slices inputs and reassembles the output.
"""

import numpy as np

import concourse.bass as bass
import concourse.mybir as mybir
import concourse.tile as tile
from concourse.masks import make_identity
from concourse.bass_utils import run_bass_kernel_spmd

# problem shape (hardcoded per harness contract)
B, S, D_IN, D_OUT, BASIS = 4, 2048, 4096, 4096, 256
N_CORES = 8
O_SHARDS, N_SHARDS = 2, 4           # grid: core = oc * N_SHARDS + nb
O_SH = D_OUT // O_SHARDS            # 2048 out-features per core
N_SH = (B * S) // N_SHARDS          # 2048 token rows per core

P = 128
KC = D_IN // P                      # 32 contraction chunks
KH = KC // 2                        # 16 chunks per D_IN half
HALF = D_IN // 2
NB = N_SH // P                      # 16 token blocks per core
NOS = O_SH // 512                   # 4 PSUM o-slices per core
OT = O_SH // P                      # 16 resid row-tiles per core
CSC = 2048.0                        # fp8 residual pre-scale

F32 = mybir.dt.float32
BF16 = mybir.dt.bfloat16
F8 = mybir.dt.float8e4
I32 = mybir.dt.int32

_WAIT_LIMIT = 1


def _split_sync_waits(nc):
    """walrus in this container rejects instructions with more than one
    embedded sync-wait command; hoist the excess onto same-engine NoOps."""
    ctr = 0
    for f in nc.m.functions:
        for bb in f.blocks:
            new = []
            changed = False
            for inst in bb.instructions:
                si = inst.sync_info
                if si is not None and si.on_wait and len(si.on_wait) > _WAIT_LIMIT:
                    waits = list(si.on_wait)
                    excess, keep = waits[:-_WAIT_LIMIT], waits[-_WAIT_LIMIT:]
                    for i in range(0, len(excess), _WAIT_LIMIT):
                        ctr += 1
                        new.append(mybir.InstNoOp(
                            name=f"I-waitsplit-{ctr}",
                            engine=inst.engine,
                            ins=[], outs=[],
                            sync_info=mybir.SyncInfo(
                                on_wait=excess[i:i + _WAIT_LIMIT], on_update=[]),
                        ))
                    si.on_wait = keep
                    changed = True
                new.append(inst)
            if changed:
                bb.instructions = new


def _build_program(split_waits=True):
    nc = bass.Bass()
    Alu = mybir.AluOpType
    Act = mybir.ActivationFunctionType
    DR = mybir.MatmulPerfMode.DoubleRow

    x_in = nc.dram_tensor("x_sh", [N_SH, D_IN], F32, kind="ExternalInput")
    codes_in = nc.dram_tensor("codes_sh", [O_SH], I32, kind="ExternalInput")
    basis_in = nc.dram_tensor("basis", [BASIS, D_IN], F32, kind="ExternalInput")
    resid_in = nc.dram_tensor("resid_sh", [O_SH, D_IN], I32, kind="ExternalInput")
    scales_in = nc.dram_tensor("scales_sh", [O_SH], F32, kind="ExternalInput")
    bias_in = nc.dram_tensor("bias_sh", [O_SH], F32, kind="ExternalInput")
    y_out = nc.dram_tensor("y_sh", [N_SH, O_SH], F32, kind="ExternalOutput")

    with tile.TileContext(nc) as tc:
        with (
            tc.tile_pool(name="const", bufs=1) as cpool,
            tc.tile_pool(name="rows", bufs=2) as rowpool,   # i32 scratch rows
            tc.tile_pool(name="xbf", bufs=3) as xbfpool,    # [128,2048] bf16
            tc.tile_pool(name="xt", bufs=4) as xtpool,      # [128,KH,128] bf16
            tc.tile_pool(name="xf8", bufs=5) as xf8pool,    # [128,KC,128] f8
            tc.tile_pool(name="zsb", bufs=2) as zsbpool,    # [128,256] bf16
            tc.tile_pool(name="r32", bufs=2) as r32pool,    # [128,HALF] i32
            tc.tile_pool(name="rbf", bufs=2) as rbfpool,    # [128,HALF] bf16
            tc.tile_pool(name="y", bufs=2) as ypool,        # [128,512] f32
            tc.tile_pool(name="psum", bufs=6, space="PSUM") as pspool,
        ):
            # ---- decode code scalars --------------------------------
            codes_row = rowpool.tile([1, O_SH], I32, tag="row",
                                     name="codes_row")
            nc.sync.dma_start(codes_row[:], codes_in[None, :])
            tmp_row = rowpool.tile([1, O_SH], I32, tag="row", name="tmp_row")
            nc.vector.tensor_scalar(tmp_row[:], codes_row[:], 0xFF, None,
                                    Alu.bitwise_and)
            idx_row_f = cpool.tile([1, O_SH], BF16, name="idx_row_f")
            nc.scalar.activation(idx_row_f[:], tmp_row[:], Act.Copy)
            rq_row = rowpool.tile([1, O_SH], I32, tag="row", name="rq_row")
            nc.vector.tensor_scalar(rq_row[:], codes_row[:], 8, 0xFFFF,
                                    Alu.logical_shift_right, Alu.bitwise_and)
            # r scaled by C so the fp8 residual path's 1/C evac matches
            r_row_f = cpool.tile([1, O_SH], BF16, name="r_row_f")
            nc.scalar.activation(r_row_f[:], rq_row[:], Act.Copy,
                                 scale=CSC / 65535.0)

            # per-row decode scale/bias for the residual (ACT layout)
            s_pp = cpool.tile([P, OT], F32, name="s_pp")
            nc.sync.dma_start(s_pp[:], scales_in.rearrange("(t p) -> p t", p=P))
            sv_pp = cpool.tile([P, OT], F32, name="sv_pp")
            nc.vector.tensor_scalar_mul(sv_pp[:], s_pp[:], CSC / 127.0)
            bv_pp = cpool.tile([P, OT], F32, name="bv_pp")
            nc.vector.tensor_scalar_mul(bv_pp[:], s_pp[:], -128.0 * CSC / 127.0)

            bias_row = cpool.tile([1, O_SH], BF16, name="bias_row")
            nc.gpsimd.dma_start(bias_row[:], bias_in[None, :])
            ones_row = cpool.tile([1, P], BF16, name="ones_row")
            nc.vector.memset(ones_row[:], 1.0)
            identity = cpool.tile([P, P], BF16, name="identity")
            make_identity(nc, identity[:])

            # ---- one-hot G [128 b, 2 bh, O_SH o] and bias broadcast --
            iota_i = cpool.tile([P, 1], I32, name="iota_i")
            nc.gpsimd.iota(iota_i[:], pattern=[[0, 1]], base=0,
                           channel_multiplier=1)
            iota_f = [cpool.tile([P, 1], F32, name=f"iota_f{bh}")
                      for bh in range(2)]
            nc.scalar.activation(iota_f[0][:], iota_i[:], Act.Copy)
            nc.scalar.activation(iota_f[1][:], iota_i[:], Act.Copy, bias=128.0,
                                 scale=1.0)
            G = cpool.tile([P, 2, O_SH], BF16, name="G")
            bias_bc = cpool.tile([P, O_SH], BF16, name="bias_bc")
            r_bc = cpool.tile([P, 512], BF16, name="r_bc")
            for q in range(NOS):
                qs = slice(q * 512, (q + 1) * 512)
                pr = pspool.tile([P, 512], F32, tag="bc", bufs=2, name=f"pr{q}")
                nc.tensor.matmul(pr[:], lhsT=ones_row[:], rhs=r_row_f[:, qs],
                                 start=True, stop=True)
                nc.scalar.copy(r_bc[:], pr[:])
                pi = pspool.tile([P, 512], F32, tag="bc", bufs=2, name=f"pi{q}")
                nc.tensor.matmul(pi[:], lhsT=ones_row[:], rhs=idx_row_f[:, qs],
                                 start=True, stop=True)
                for bh in range(2):
                    nc.vector.scalar_tensor_tensor(
                        G[:, bh, qs], pi[:], iota_f[bh][:, :1], r_bc[:],
                        op0=Alu.is_equal, op1=Alu.mult)
                pb = pspool.tile([P, 512], F32, tag="bc", bufs=2, name=f"pb{q}")
                nc.tensor.matmul(pb[:], lhsT=ones_row[:], rhs=bias_row[:, qs],
                                 start=True, stop=True)
                nc.scalar.copy(bias_bc[:, qs], pb[:])

            # ---- basisT [128 i, KC k, 256 b] bf16 --------------------
            basisT = cpool.tile([P, KC, BASIS], BF16, name="basisT")
            for bh2 in range(2):
                for hf in range(2):
                    hs = slice(hf * HALF, (hf + 1) * HALF)
                    b_bf = xbfpool.tile([P, HALF], BF16, tag="xbf",
                                        name=f"bbf{bh2}_{hf}")
                    nc.gpsimd.dma_start(b_bf[:],
                                        basis_in[bh2 * P:(bh2 + 1) * P, hs])
                    nc.sync.dma_start_transpose(
                        basisT[:, hf * KH:(hf + 1) * KH, bh2 * P:(bh2 + 1) * P],
                        b_bf[:])

            # ---- persistent W^T (residual, fp8) and zT ---------------
            residT = cpool.tile([P, KC, O_SH], F8, name="residT")
            zT = cpool.tile([P, 2, N_SH], BF16, name="zT")

            def resid_step(t, hf):
                # [128 o-rows, 2048 i-half]: load, decode, transpose, cast
                hs = slice(hf * HALF, (hf + 1) * HALF)
                eng_a = nc.scalar if (2 * t + hf) % 2 == 0 else nc.gpsimd
                rbf = rbfpool.tile([P, HALF], BF16, tag="rbf",
                                   name=f"rbf{t}_{hf}")
                for qh in range(2):
                    qsl = slice(qh * 1024, (qh + 1) * 1024)
                    qsrc = slice(hf * HALF + qh * 1024,
                                 hf * HALF + (qh + 1) * 1024)
                    r32 = r32pool.tile([P, 1024], I32, tag="r32",
                                       name=f"r32_{t}_{hf}_{qh}")
                    eng_a.dma_start(r32[:], resid_in[t * P:(t + 1) * P, qsrc])
                    if (2 * t + hf) % 2 == 0:
                        nc.scalar.activation(rbf[:, qsl], r32[:], Act.Identity,
                                             bias=bv_pp[:, t:t + 1],
                                             scale=sv_pp[:, t:t + 1])
                    else:
                        nc.vector.tensor_scalar(rbf[:, qsl], r32[:],
                                                sv_pp[:, t:t + 1],
                                                bv_pp[:, t:t + 1],
                                                Alu.mult, Alu.add)
                # PE transpose (xbar is descriptor-bound); evac casts to f8
                for g in range(4):
                    prt = pspool.tile([P, 512], BF16, tag="bc", bufs=2,
                                      name=f"prt{t}_{hf}_{g}")
                    for j in range(4):
                        kk = 4 * g + j
                        nc.tensor.transpose(
                            prt[:, j * P:(j + 1) * P],
                            rbf[:, kk * P:(kk + 1) * P], identity[:])
                    k0 = hf * KH + 4 * g
                    ceng = nc.scalar if (t + g) % 2 == 0 else nc.vector
                    if ceng is nc.scalar:
                        nc.scalar.copy(
                            residT[:, k0:k0 + 4, t * P:(t + 1) * P], prt[:])
                    else:
                        nc.vector.tensor_copy(
                            residT[:, k0:k0 + 4, t * P:(t + 1) * P], prt[:])

            def xz_block(tb):
                # one token block: x -> xT (bf16+f8), z chain, zT via PE
                # each xbar writes its own full tile (concurrent sliced
                # transpose-writes to one tile race on hardware)
                xth = []
                for hf in range(2):
                    hs = slice(hf * HALF, (hf + 1) * HALF)
                    xbf = xbfpool.tile([P, HALF], BF16, tag="xbf",
                                       name=f"xbf{tb}_{hf}")
                    nc.gpsimd.dma_start(xbf[:], x_in[tb * P:(tb + 1) * P, hs])
                    xt = xtpool.tile([P, KH, P], BF16, tag="xt",
                                     name=f"xt{tb}_{hf}")
                    nc.sync.dma_start_transpose(xt[:], xbf[:])
                    xth.append(xt)
                xf8 = xf8pool.tile([P, KC, P], F8, tag="xf8", name=f"xf8_{tb}")
                for hf in range(2):
                    nc.vector.tensor_copy(
                        xf8[:, hf * KH:(hf + 1) * KH, :], xth[hf][:])
                # z[tok, b] with full-width rhs, then PE-transpose to zT
                pz = pspool.tile([P, 512], F32, tag="z", bufs=2,
                                 name=f"pz{tb}")
                for k in range(KC):
                    nc.tensor.matmul(pz[:, :2 * P], lhsT=xth[k // KH][:, k % KH, :],
                                     rhs=basisT[:, k, :],
                                     start=(k == 0), stop=(k == KC - 1))
                z_sb = zsbpool.tile([P, 2 * P], BF16, tag="z", name=f"zsb{tb}")
                nc.scalar.copy(z_sb[:], pz[:, :2 * P])
                for bh in range(2):
                    # full-bank PSUM tile: sub-bank tiles corrupt each
                    # other through the bank-granular start=True zeroing
                    pzT = pspool.tile([P, 1024], BF16, tag="zt", bufs=1,
                                      name=f"pzT{tb}_{bh}")
                    nc.tensor.transpose(pzT[:, :P], z_sb[:, bh * P:(bh + 1) * P],
                                        identity[:])
                    nc.scalar.copy(zT[:, bh, tb * P:(tb + 1) * P], pzT[:, :P])
                return xf8

            xf8s = [None] * NB

            def main_mm(tb, os):
                osl = slice(os * 512, (os + 1) * 512)
                tbs = slice(tb * P, (tb + 1) * P)
                xf8 = xf8s[tb]
                ps = pspool.tile([P, 512], F32, tag="mm", bufs=3,
                                 name=f"ps{tb}_{os}")
                for bh in range(2):
                    nc.tensor.matmul(ps[:], lhsT=zT[:, bh, tbs],
                                     rhs=G[:, bh, osl],
                                     start=(bh == 0), stop=False)
                for k2 in range(KH):
                    ks = slice(2 * k2, 2 * k2 + 2)
                    nc.tensor.matmul(ps[:], lhsT=xf8[:, ks, :],
                                     rhs=residT[:, ks, osl],
                                     start=False, stop=(k2 == KH - 1),
                                     perf_mode=DR)
                y_t = ypool.tile([P, 512], F32, tag="y", name=f"y{tb}_{os}")
                nc.vector.scalar_tensor_tensor(y_t[:], ps[:], 1.0 / CSC,
                                               bias_bc[:, osl],
                                               op0=Alu.mult, op1=Alu.add)
                nc.scalar.dma_start(y_out[tbs, osl], y_t[:])

            # ---- interleaved schedule (issue order = engine order) ---
            for t in (0, 1, 2, 3):
                resid_step(t, 0), resid_step(t, 1)
            xf8s[0] = xz_block(0)
            xf8s[1] = xz_block(1)
            for t in (4, 5):
                resid_step(t, 0), resid_step(t, 1)
            xf8s[2] = xz_block(2)
            xf8s[3] = xz_block(3)
            for t in (6, 7):
                resid_step(t, 0), resid_step(t, 1)
            main_mm(0, 0), main_mm(1, 0)
            for t in (8, 9):
                resid_step(t, 0), resid_step(t, 1)
            main_mm(2, 0), main_mm(3, 0)
            for t in (10, 11):
                resid_step(t, 0), resid_step(t, 1)
            main_mm(0, 1), main_mm(1, 1)
            for t in (12, 13):
                resid_step(t, 0), resid_step(t, 1)
            main_mm(2, 1), main_mm(3, 1)
            for t in (14, 15):
                resid_step(t, 0), resid_step(t, 1)
            for tb in range(4):
                main_mm(tb, 2)
            for tb in range(4):
                main_mm(tb, 3)
            xf8s[4] = xz_block(4)
            for tb in range(4, NB):
                if tb + 1 < NB:
                    xf8s[tb + 1] = xz_block(tb + 1)
                for os in range(NOS):
                    main_mm(tb, os)

    if split_waits:
        _split_sync_waits(nc)
    return nc


_program_cache = {}


def _get_program():
    if "nc" not in _program_cache:
        _program_cache["nc"] = _build_program()
    return _program_cache["nc"]


def _in_maps(x, codes, basis_table, residual_q, residual_scales, bias):
    x2 = x.reshape(B * S, D_IN)
    in_maps = []
    for core in range(N_CORES):
        oc, nb = divmod(core, N_SHARDS)
        osl = slice(oc * O_SH, (oc + 1) * O_SH)
        nsl = slice(nb * N_SH, (nb + 1) * N_SH)
        in_maps.append({
            "x_sh": np.ascontiguousarray(x2[nsl]),
            "codes_sh": np.ascontiguousarray(codes[osl]),
            "basis": basis_table,
            "resid_sh": np.ascontiguousarray(residual_q[osl]),
            "scales_sh": np.ascontiguousarray(residual_scales[osl]),
            "bias_sh": np.ascontiguousarray(bias[osl]),
        })
    return in_maps


def kernel(x, codes, basis_table, residual_q, residual_scales, bias):
    x = np.ascontiguousarray(np.asarray(x, dtype=np.float32))
    codes = np.ascontiguousarray(np.asarray(codes, dtype=np.int32))
    basis_table = np.ascontiguousarray(np.asarray(basis_table, dtype=np.float32))
    residual_q = np.ascontiguousarray(np.asarray(residual_q, dtype=np.int32))
    residual_scales = np.ascontiguousarray(
        np.asarray(residual_scales, dtype=np.float32))
    bias = np.ascontiguousarray(np.asarray(bias, dtype=np.float32))

    nc = _get_program()
    res = run_bass_kernel_spmd(nc, _in_maps(x, codes, basis_table, residual_q,
                                            residual_scales, bias),
                               core_ids=list(range(N_CORES)))

    y = np.empty((B * S, D_OUT), dtype=np.float32)
    for core in range(N_CORES):
        oc, nb = divmod(core, N_SHARDS)
        y[nb * N_SH:(nb + 1) * N_SH, oc * O_SH:(oc + 1) * O_SH] = \
            res.results[core]["y_sh"]
    return y.reshape(B, S, D_OUT)


# revision 13
# speedup vs baseline: 1.3202x; 1.0940x over previous
"""BitfieldLinear (vq_codebook) Trainium2 kernel — fp8 decomposed.

y = x @ W^T + bias, W = r[:,None]*basis[idx] + s[:,None]*(q-128)/127.

Instead of materializing W in bf16 (PE floor ~437us/core), split:
  y = zT.T @ G + x_f8 @ residT_f8 * (1/C) + bias
  - z = x @ basisT in bf16 ([tokens, 256 basis]); the basis term
    becomes a one-hot gather matmul with G[b,o] = C*r[o] at b=idx[o]
    (accurate path, 1/8 the flops).
  - the residual matmul runs in fp8e4 DoubleRow perf mode (2x PE rate);
    residT_f8 pre-scaled by s*C/127 (C=2048 keeps values out of the
    fp8 subnormal range), x cast bf16->fp8 directly.
  Both accumulate into the same PSUM bank; evacuation is one DVE op
  (psum * 1/C + bias_bc).

Sharding across 8 NeuronCores: 2-way over out_features x 4-way over
flattened tokens. Per core: 2048 tokens x 2048 outs, K=4096.
  - resid rows stream: int32 DMA -> ACT/DVE decode (s*C/127 folded,
    bf16) -> xbar transpose -> DVE cast to residT_f8 [128,32k,2048o].
  - x streams via gpsimd cast-DMA (f32->bf16) -> xbar -> fp8 cast;
    z chains + PE transposes of z run during the resid build window.
Host only slices inputs and reassembles the output.
"""

import numpy as np

import concourse.bass as bass
import concourse.mybir as mybir
import concourse.tile as tile
from concourse.masks import make_identity
from concourse.bass_utils import run_bass_kernel_spmd

# problem shape (hardcoded per harness contract)
B, S, D_IN, D_OUT, BASIS = 4, 2048, 4096, 4096, 256
N_CORES = 8
O_SHARDS, N_SHARDS = 2, 4           # grid: core = oc * N_SHARDS + nb
O_SH = D_OUT // O_SHARDS            # 2048 out-features per core
N_SH = (B * S) // N_SHARDS          # 2048 token rows per core

P = 128
KC = D_IN // P                      # 32 contraction chunks
KH = KC // 2                        # 16 chunks per D_IN half
HALF = D_IN // 2
NB = N_SH // P                      # 16 token blocks per core
NOS = O_SH // 512                   # 4 PSUM o-slices per core
OT = O_SH // P                      # 16 resid row-tiles per core
CSC = 2048.0                        # fp8 residual pre-scale

F32 = mybir.dt.float32
BF16 = mybir.dt.bfloat16
F8 = mybir.dt.float8e4
I32 = mybir.dt.int32

_WAIT_LIMIT = 1


def _split_sync_waits(nc):
    """walrus in this container rejects instructions with more than one
    embedded sync-wait command; hoist the excess onto same-engine NoOps."""
    ctr = 0
    for f in nc.m.functions:
        for bb in f.blocks:
            new = []
            changed = False
            for inst in bb.instructions:
                si = inst.sync_info
                if si is not None and si.on_wait and len(si.on_wait) > _WAIT_LIMIT:
                    waits = list(si.on_wait)
                    excess, keep = waits[:-_WAIT_LIMIT], waits[-_WAIT_LIMIT:]
                    for i in range(0, len(excess), _WAIT_LIMIT):
                        ctr += 1
                        new.append(mybir.InstNoOp(
                            name=f"I-waitsplit-{ctr}",
                            engine=inst.engine,
                            ins=[], outs=[],
                            sync_info=mybir.SyncInfo(
                                on_wait=excess[i:i + _WAIT_LIMIT], on_update=[]),
                        ))
                    si.on_wait = keep
                    changed = True
                new.append(inst)
            if changed:
                bb.instructions = new


def _build_program(split_waits=True):
    nc = bass.Bass()
    Alu = mybir.AluOpType
    Act = mybir.ActivationFunctionType
    DR = mybir.MatmulPerfMode.DoubleRow

    x_in = nc.dram_tensor("x_sh", [N_SH, D_IN], F32, kind="ExternalInput")
    codes_in = nc.dram_tensor("codes_sh", [O_SH], I32, kind="ExternalInput")
    basis_in = nc.dram_tensor("basis", [BASIS, D_IN], F32, kind="ExternalInput")
    resid_in = nc.dram_tensor("resid_sh", [O_SH, D_IN], I32, kind="ExternalInput")
    scales_in = nc.dram_tensor("scales_sh", [O_SH], F32, kind="ExternalInput")
    bias_in = nc.dram_tensor("bias_sh", [O_SH], F32, kind="ExternalInput")
    y_out = nc.dram_tensor("y_sh", [N_SH, O_SH], F32, kind="ExternalOutput")

    with tile.TileContext(nc) as tc:
        with (
            tc.tile_pool(name="const", bufs=1) as cpool,
            tc.tile_pool(name="rows", bufs=2) as rowpool,   # i32 scratch rows
            tc.tile_pool(name="xbf", bufs=2) as xbfpool,    # [128,4096] bf16
            tc.tile_pool(name="xt", bufs=2) as xtpool,      # [128,KC,128] bf16
            tc.tile_pool(name="xf8", bufs=4) as xf8pool,    # [128,KC,128] f8
            tc.tile_pool(name="zsb", bufs=2) as zsbpool,    # [128,256] bf16
            tc.tile_pool(name="r32", bufs=2) as r32pool,    # [128,HALF] i32
            tc.tile_pool(name="rbf", bufs=2) as rbfpool,    # [128,HALF] bf16
            tc.tile_pool(name="y", bufs=2) as ypool,        # [128,512] f32
            tc.tile_pool(name="psum", bufs=6, space="PSUM") as pspool,
        ):
            # ---- decode code scalars --------------------------------
            codes_row = rowpool.tile([1, O_SH], I32, tag="row",
                                     name="codes_row")
            nc.sync.dma_start(codes_row[:], codes_in[None, :])
            tmp_row = rowpool.tile([1, O_SH], I32, tag="row", name="tmp_row")
            nc.vector.tensor_scalar(tmp_row[:], codes_row[:], 0xFF, None,
                                    Alu.bitwise_and)
            idx_row_f = cpool.tile([1, O_SH], BF16, name="idx_row_f")
            nc.scalar.activation(idx_row_f[:], tmp_row[:], Act.Copy)
            rq_row = rowpool.tile([1, O_SH], I32, tag="row", name="rq_row")
            nc.vector.tensor_scalar(rq_row[:], codes_row[:], 8, 0xFFFF,
                                    Alu.logical_shift_right, Alu.bitwise_and)
            # r scaled by C so the fp8 residual path's 1/C evac matches
            r_row_f = cpool.tile([1, O_SH], BF16, name="r_row_f")
            nc.scalar.activation(r_row_f[:], rq_row[:], Act.Copy,
                                 scale=CSC / 65535.0)

            # per-row decode scale/bias for the residual (ACT layout)
            s_pp = cpool.tile([P, OT], F32, name="s_pp")
            nc.sync.dma_start(s_pp[:], scales_in.rearrange("(t p) -> p t", p=P))
            sv_pp = cpool.tile([P, OT], F32, name="sv_pp")
            nc.vector.tensor_scalar_mul(sv_pp[:], s_pp[:], CSC / 127.0)
            bv_pp = cpool.tile([P, OT], F32, name="bv_pp")
            nc.vector.tensor_scalar_mul(bv_pp[:], s_pp[:], -128.0 * CSC / 127.0)

            bias_row = cpool.tile([1, O_SH], BF16, name="bias_row")
            nc.gpsimd.dma_start(bias_row[:], bias_in[None, :])
            ones_row = cpool.tile([1, P], BF16, name="ones_row")
            nc.vector.memset(ones_row[:], 1.0)
            identity = cpool.tile([P, P], BF16, name="identity")
            make_identity(nc, identity[:])

            # ---- one-hot G [128 b, 2 bh, O_SH o] and bias broadcast --
            iota_i = cpool.tile([P, 1], I32, name="iota_i")
            nc.gpsimd.iota(iota_i[:], pattern=[[0, 1]], base=0,
                           channel_multiplier=1)
            iota_f = [cpool.tile([P, 1], F32, name=f"iota_f{bh}")
                      for bh in range(2)]
            nc.scalar.activation(iota_f[0][:], iota_i[:], Act.Copy)
            nc.scalar.activation(iota_f[1][:], iota_i[:], Act.Copy, bias=128.0,
                                 scale=1.0)
            G = cpool.tile([P, 2, O_SH], BF16, name="G")
            bias_bc = cpool.tile([P, O_SH], BF16, name="bias_bc")
            r_bc = cpool.tile([P, 512], BF16, name="r_bc")
            for q in range(NOS):
                qs = slice(q * 512, (q + 1) * 512)
                pr = pspool.tile([P, 512], F32, tag="bc", bufs=2, name=f"pr{q}")
                nc.tensor.matmul(pr[:], lhsT=ones_row[:], rhs=r_row_f[:, qs],
                                 start=True, stop=True)
                nc.scalar.copy(r_bc[:], pr[:])
                pi = pspool.tile([P, 512], F32, tag="bc", bufs=2, name=f"pi{q}")
                nc.tensor.matmul(pi[:], lhsT=ones_row[:], rhs=idx_row_f[:, qs],
                                 start=True, stop=True)
                for bh in range(2):
                    nc.vector.scalar_tensor_tensor(
                        G[:, bh, qs], pi[:], iota_f[bh][:, :1], r_bc[:],
                        op0=Alu.is_equal, op1=Alu.mult)
                pb = pspool.tile([P, 512], F32, tag="bc", bufs=2, name=f"pb{q}")
                nc.tensor.matmul(pb[:], lhsT=ones_row[:], rhs=bias_row[:, qs],
                                 start=True, stop=True)
                nc.scalar.copy(bias_bc[:, qs], pb[:])

            # ---- basisT [128 i, KC k, 256 b] bf16 --------------------
            basisT = cpool.tile([P, KC, BASIS], BF16, name="basisT")
            for bh2 in range(2):
                b_bf = xbfpool.tile([P, D_IN], BF16, tag="xbf",
                                    name=f"bbf{bh2}")
                nc.gpsimd.dma_start(b_bf[:], basis_in[bh2 * P:(bh2 + 1) * P, :])
                nc.sync.dma_start_transpose(
                    basisT[:, :, bh2 * P:(bh2 + 1) * P], b_bf[:])

            # ---- persistent W^T (residual, fp8) and zT ---------------
            residT = cpool.tile([P, KC, O_SH], F8, name="residT")
            zT = cpool.tile([P, 2, N_SH], BF16, name="zT")

            def resid_step(t, hf):
                # [128 o-rows, 2048 i-half]: load, decode, transpose, cast
                hs = slice(hf * HALF, (hf + 1) * HALF)
                eng_a = nc.scalar
                rbf = rbfpool.tile([P, HALF], BF16, tag="rbf",
                                   name=f"rbf{t}_{hf}")
                for qh in range(2):
                    qsl = slice(qh * 1024, (qh + 1) * 1024)
                    qsrc = slice(hf * HALF + qh * 1024,
                                 hf * HALF + (qh + 1) * 1024)
                    r32 = r32pool.tile([P, 1024], I32, tag="r32",
                                       name=f"r32_{t}_{hf}_{qh}")
                    eng_a.dma_start(r32[:], resid_in[t * P:(t + 1) * P, qsrc])
                    if (2 * t + hf) % 2 == 0:
                        nc.scalar.activation(rbf[:, qsl], r32[:], Act.Identity,
                                             bias=bv_pp[:, t:t + 1],
                                             scale=sv_pp[:, t:t + 1])
                    else:
                        nc.vector.tensor_scalar(rbf[:, qsl], r32[:],
                                                sv_pp[:, t:t + 1],
                                                bv_pp[:, t:t + 1],
                                                Alu.mult, Alu.add)
                # PE transpose (xbar is descriptor-bound); evac casts to f8
                for g in range(4):
                    prt = pspool.tile([P, 512], BF16, tag="bc", bufs=2,
                                      name=f"prt{t}_{hf}_{g}")
                    for j in range(4):
                        kk = 4 * g + j
                        nc.tensor.transpose(
                            prt[:, j * P:(j + 1) * P],
                            rbf[:, kk * P:(kk + 1) * P], identity[:])
                    k0 = hf * KH + 4 * g
                    ceng = nc.scalar if (t + g) % 2 == 0 else nc.vector
                    if ceng is nc.scalar:
                        nc.scalar.copy(
                            residT[:, k0:k0 + 4, t * P:(t + 1) * P], prt[:])
                    else:
                        nc.vector.tensor_copy(
                            residT[:, k0:k0 + 4, t * P:(t + 1) * P], prt[:])

            def xz_block(tb):
                # one token block: x -> xT (bf16+f8), z chain, zT via PE
                xbf = xbfpool.tile([P, D_IN], BF16, tag="xbf",
                                   name=f"xbf{tb}")
                nc.gpsimd.dma_start(xbf[:], x_in[tb * P:(tb + 1) * P, :])
                xt = xtpool.tile([P, KC, P], BF16, tag="xt", name=f"xt{tb}")
                nc.sync.dma_start_transpose(xt[:], xbf[:])
                xf8 = xf8pool.tile([P, KC, P], F8, tag="xf8", name=f"xf8_{tb}")
                nc.vector.tensor_copy(xf8[:], xt[:])
                # z[tok, b] with full-width rhs, then PE-transpose to zT
                pz = pspool.tile([P, 512], F32, tag="z", bufs=2,
                                 name=f"pz{tb}")
                for k in range(KC):
                    nc.tensor.matmul(pz[:, :2 * P], lhsT=xt[:, k, :],
                                     rhs=basisT[:, k, :],
                                     start=(k == 0), stop=(k == KC - 1))
                z_sb = zsbpool.tile([P, 2 * P], BF16, tag="z", name=f"zsb{tb}")
                nc.scalar.copy(z_sb[:], pz[:, :2 * P])
                for bh in range(2):
                    # full-bank PSUM tile: sub-bank tiles corrupt each
                    # other through the bank-granular start=True zeroing
                    pzT = pspool.tile([P, 1024], BF16, tag="zt", bufs=1,
                                      name=f"pzT{tb}_{bh}")
                    nc.tensor.transpose(pzT[:, :P], z_sb[:, bh * P:(bh + 1) * P],
                                        identity[:])
                    nc.scalar.copy(zT[:, bh, tb * P:(tb + 1) * P], pzT[:, :P])
                return xf8

            xf8s = [None] * NB

            def main_mm(tb, os):
                osl = slice(os * 512, (os + 1) * 512)
                tbs = slice(tb * P, (tb + 1) * P)
                xf8 = xf8s[tb]
                ps = pspool.tile([P, 512], F32, tag="mm", bufs=3,
                                 name=f"ps{tb}_{os}")
                for bh in range(2):
                    nc.tensor.matmul(ps[:], lhsT=zT[:, bh, tbs],
                                     rhs=G[:, bh, osl],
                                     start=(bh == 0), stop=False)
                for k2 in range(KH):
                    ks = slice(2 * k2, 2 * k2 + 2)
                    nc.tensor.matmul(ps[:], lhsT=xf8[:, ks, :],
                                     rhs=residT[:, ks, osl],
                                     start=False, stop=(k2 == KH - 1),
                                     perf_mode=DR)
                y_t = ypool.tile([P, 512], F32, tag="y", name=f"y{tb}_{os}")
                nc.vector.scalar_tensor_tensor(y_t[:], ps[:], 1.0 / CSC,
                                               bias_bc[:, osl],
                                               op0=Alu.mult, op1=Alu.add)
                nc.scalar.dma_start(y_out[tbs, osl], y_t[:])

            # ---- interleaved schedule (issue order = engine order) ---
            for t in (0, 1, 2, 3):
                resid_step(t, 0), resid_step(t, 1)
            xf8s[0] = xz_block(0)
            xf8s[1] = xz_block(1)
            for t in (4, 5):
                resid_step(t, 0), resid_step(t, 1)
            xf8s[2] = xz_block(2)
            xf8s[3] = xz_block(3)
            for t in (6, 7):
                resid_step(t, 0), resid_step(t, 1)
            main_mm(0, 0), main_mm(1, 0)
            for t in (8, 9):
                resid_step(t, 0), resid_step(t, 1)
            main_mm(2, 0), main_mm(3, 0)
            for t in (10, 11):
                resid_step(t, 0), resid_step(t, 1)
            main_mm(0, 1), main_mm(1, 1)
            for t in (12, 13):
                resid_step(t, 0), resid_step(t, 1)
            main_mm(2, 1), main_mm(3, 1)
            for t in (14, 15):
                resid_step(t, 0), resid_step(t, 1)
            for tb in range(4):
                main_mm(tb, 2)
            for tb in range(4):
                main_mm(tb, 3)
            xf8s[4] = xz_block(4)
            for tb in range(4, NB):
                if tb + 1 < NB:
                    xf8s[tb + 1] = xz_block(tb + 1)
                for os in range(NOS):
                    main_mm(tb, os)

    if split_waits:
        _split_sync_waits(nc)
    return nc


_program_cache = {}


def _get_program():
    if "nc" not in _program_cache:
        _program_cache["nc"] = _build_program()
    return _program_cache["nc"]


def _in_maps(x, codes, basis_table, residual_q, residual_scales, bias):
    x2 = x.reshape(B * S, D_IN)
    in_maps = []
    for core in range(N_CORES):
        oc, nb = divmod(core, N_SHARDS)
        osl = slice(oc * O_SH, (oc + 1) * O_SH)
        nsl = slice(nb * N_SH, (nb + 1) * N_SH)
        in_maps.append({
            "x_sh": np.ascontiguousarray(x2[nsl]),
            "codes_sh": np.ascontiguousarray(codes[osl]),
            "basis": basis_table,
            "resid_sh": np.ascontiguousarray(residual_q[osl]),
            "scales_sh": np.ascontiguousarray(residual_scales[osl]),
            "bias_sh": np.ascontiguousarray(bias[osl]),
        })
    return in_maps


def kernel(x, codes, basis_table, residual_q, residual_scales, bias):
    x = np.ascontiguousarray(np.asarray(x, dtype=np.float32))
    codes = np.ascontiguousarray(np.asarray(codes, dtype=np.int32))
    basis_table = np.ascontiguousarray(np.asarray(basis_table, dtype=np.float32))
    residual_q = np.ascontiguousarray(np.asarray(residual_q, dtype=np.int32))
    residual_scales = np.ascontiguousarray(
        np.asarray(residual_scales, dtype=np.float32))
    bias = np.ascontiguousarray(np.asarray(bias, dtype=np.float32))

    nc = _get_program()
    res = run_bass_kernel_spmd(nc, _in_maps(x, codes, basis_table, residual_q,
                                            residual_scales, bias),
                               core_ids=list(range(N_CORES)))

    y = np.empty((B * S, D_OUT), dtype=np.float32)
    for core in range(N_CORES):
        oc, nb = divmod(core, N_SHARDS)
        y[nb * N_SH:(nb + 1) * N_SH, oc * O_SH:(oc + 1) * O_SH] = \
            res.results[core]["y_sh"]
    return y.reshape(B, S, D_OUT)


# revision 14
# speedup vs baseline: 1.4304x; 1.0834x over previous
"""BitfieldLinear (vq_codebook) Trainium2 kernel — fp8 decomposed.

y = x @ W^T + bias, W = r[:,None]*basis[idx] + s[:,None]*(q-128)/127.

Instead of materializing W in bf16 (PE floor ~437us/core), split:
  y = zT.T @ G + x_f8 @ residT_f8 * (1/C) + bias
  - z = x @ basisT in bf16 ([tokens, 256 basis]); the basis term
    becomes a one-hot gather matmul with G[b,o] = C*r[o] at b=idx[o]
    (accurate path, 1/8 the flops).
  - the residual matmul runs in fp8e4 DoubleRow perf mode (2x PE rate);
    residT_f8 pre-scaled by s*C/127 (C=2048 keeps values out of the
    fp8 subnormal range), x cast bf16->fp8 directly.
  Both accumulate into the same PSUM bank; evacuation is one DVE op
  (psum * 1/C + bias_bc).

Sharding across 8 NeuronCores: 2-way over out_features x 4-way over
flattened tokens. Per core: 2048 tokens x 2048 outs, K=4096.
  - resid rows stream: int32 DMA -> ACT/DVE decode (s*C/127 folded,
    bf16) -> xbar transpose -> DVE cast to residT_f8 [128,32k,2048o].
  - x streams via gpsimd cast-DMA (f32->bf16) -> xbar -> fp8 cast;
    z chains + PE transposes of z run during the resid build window.
Host only slices inputs and reassembles the output.
"""

import numpy as np

import concourse.bass as bass
import concourse.mybir as mybir
import concourse.tile as tile
from concourse.masks import make_identity
from concourse.bass_utils import run_bass_kernel_spmd

# problem shape (hardcoded per harness contract)
B, S, D_IN, D_OUT, BASIS = 4, 2048, 4096, 4096, 256
N_CORES = 8
O_SHARDS, N_SHARDS = 2, 4           # grid: core = oc * N_SHARDS + nb
O_SH = D_OUT // O_SHARDS            # 2048 out-features per core
N_SH = (B * S) // N_SHARDS          # 2048 token rows per core

P = 128
KC = D_IN // P                      # 32 contraction chunks
KH = KC // 2                        # 16 chunks per D_IN half
HALF = D_IN // 2
NB = N_SH // P                      # 16 token blocks per core
NOS = O_SH // 512                   # 4 PSUM o-slices per core
OT = O_SH // P                      # 16 resid row-tiles per core
CSC = 2048.0                        # fp8 residual pre-scale

F32 = mybir.dt.float32
BF16 = mybir.dt.bfloat16
F8 = mybir.dt.float8e4
I32 = mybir.dt.int32

_WAIT_LIMIT = 1


def _split_sync_waits(nc):
    """walrus in this container rejects instructions with more than one
    embedded sync-wait command; hoist the excess onto same-engine NoOps."""
    ctr = 0
    for f in nc.m.functions:
        for bb in f.blocks:
            new = []
            changed = False
            for inst in bb.instructions:
                si = inst.sync_info
                if si is not None and si.on_wait and len(si.on_wait) > _WAIT_LIMIT:
                    waits = list(si.on_wait)
                    excess, keep = waits[:-_WAIT_LIMIT], waits[-_WAIT_LIMIT:]
                    for i in range(0, len(excess), _WAIT_LIMIT):
                        ctr += 1
                        new.append(mybir.InstNoOp(
                            name=f"I-waitsplit-{ctr}",
                            engine=inst.engine,
                            ins=[], outs=[],
                            sync_info=mybir.SyncInfo(
                                on_wait=excess[i:i + _WAIT_LIMIT], on_update=[]),
                        ))
                    si.on_wait = keep
                    changed = True
                new.append(inst)
            if changed:
                bb.instructions = new


def _build_program(split_waits=True):
    nc = bass.Bass()
    Alu = mybir.AluOpType
    Act = mybir.ActivationFunctionType
    DR = mybir.MatmulPerfMode.DoubleRow

    x_in = nc.dram_tensor("x_sh", [N_SH, D_IN], F32, kind="ExternalInput")
    codes_in = nc.dram_tensor("codes_sh", [O_SH], I32, kind="ExternalInput")
    basis_in = nc.dram_tensor("basis", [BASIS, D_IN], F32, kind="ExternalInput")
    resid_in = nc.dram_tensor("resid_sh", [O_SH, D_IN], I32, kind="ExternalInput")
    scales_in = nc.dram_tensor("scales_sh", [O_SH], F32, kind="ExternalInput")
    bias_in = nc.dram_tensor("bias_sh", [O_SH], F32, kind="ExternalInput")
    y_out = nc.dram_tensor("y_sh", [N_SH, O_SH], F32, kind="ExternalOutput")

    with tile.TileContext(nc) as tc:
        with (
            tc.tile_pool(name="const", bufs=1) as cpool,
            tc.tile_pool(name="rows", bufs=2) as rowpool,   # i32 scratch rows
            tc.tile_pool(name="xbf", bufs=2) as xbfpool,    # [128,4096] bf16
            tc.tile_pool(name="xt", bufs=2) as xtpool,      # [128,KC,128] bf16
            tc.tile_pool(name="xf8", bufs=4) as xf8pool,    # [128,KC,128] f8
            tc.tile_pool(name="zsb", bufs=2) as zsbpool,    # [128,256] bf16
            tc.tile_pool(name="r32", bufs=2) as r32pool,    # [128,2048] i32
            tc.tile_pool(name="rbf", bufs=2) as rbfpool,    # [128,HALF] bf16
            tc.tile_pool(name="y", bufs=2) as ypool,        # [128,512] f32
            tc.tile_pool(name="psum", bufs=6, space="PSUM") as pspool,
        ):
            # ---- decode code scalars --------------------------------
            codes_row = rowpool.tile([1, O_SH], I32, tag="row",
                                     name="codes_row")
            nc.sync.dma_start(codes_row[:], codes_in[None, :])
            tmp_row = rowpool.tile([1, O_SH], I32, tag="row", name="tmp_row")
            nc.vector.tensor_scalar(tmp_row[:], codes_row[:], 0xFF, None,
                                    Alu.bitwise_and)
            idx_row_f = cpool.tile([1, O_SH], BF16, name="idx_row_f")
            nc.scalar.activation(idx_row_f[:], tmp_row[:], Act.Copy)
            rq_row = rowpool.tile([1, O_SH], I32, tag="row", name="rq_row")
            nc.vector.tensor_scalar(rq_row[:], codes_row[:], 8, 0xFFFF,
                                    Alu.logical_shift_right, Alu.bitwise_and)
            # r scaled by C so the fp8 residual path's 1/C evac matches
            r_row_f = cpool.tile([1, O_SH], BF16, name="r_row_f")
            nc.scalar.activation(r_row_f[:], rq_row[:], Act.Copy,
                                 scale=CSC / 65535.0)

            # per-row decode scale/bias for the residual (ACT layout)
            s_pp = cpool.tile([P, OT], F32, name="s_pp")
            nc.sync.dma_start(s_pp[:], scales_in.rearrange("(t p) -> p t", p=P))
            sv_pp = cpool.tile([P, OT], F32, name="sv_pp")
            nc.vector.tensor_scalar_mul(sv_pp[:], s_pp[:], CSC / 127.0)
            bv_pp = cpool.tile([P, OT], F32, name="bv_pp")
            nc.vector.tensor_scalar_mul(bv_pp[:], s_pp[:], -128.0 * CSC / 127.0)

            bias_row = cpool.tile([1, O_SH], BF16, name="bias_row")
            nc.gpsimd.dma_start(bias_row[:], bias_in[None, :])
            ones_row = cpool.tile([1, P], BF16, name="ones_row")
            nc.vector.memset(ones_row[:], 1.0)
            identity = cpool.tile([P, P], BF16, name="identity")
            make_identity(nc, identity[:])

            # ---- one-hot G [128 b, 2 bh, O_SH o] and bias broadcast --
            iota_i = cpool.tile([P, 1], I32, name="iota_i")
            nc.gpsimd.iota(iota_i[:], pattern=[[0, 1]], base=0,
                           channel_multiplier=1)
            iota_f = [cpool.tile([P, 1], F32, name=f"iota_f{bh}")
                      for bh in range(2)]
            nc.scalar.activation(iota_f[0][:], iota_i[:], Act.Copy)
            nc.scalar.activation(iota_f[1][:], iota_i[:], Act.Copy, bias=128.0,
                                 scale=1.0)
            G = cpool.tile([P, 2, O_SH], BF16, name="G")
            bias_bc = cpool.tile([P, O_SH], BF16, name="bias_bc")
            r_bc = cpool.tile([P, 512], BF16, name="r_bc")
            for q in range(NOS):
                qs = slice(q * 512, (q + 1) * 512)
                pr = pspool.tile([P, 512], F32, tag="bc", bufs=2, name=f"pr{q}")
                nc.tensor.matmul(pr[:], lhsT=ones_row[:], rhs=r_row_f[:, qs],
                                 start=True, stop=True)
                nc.scalar.copy(r_bc[:], pr[:])
                pi = pspool.tile([P, 512], F32, tag="bc", bufs=2, name=f"pi{q}")
                nc.tensor.matmul(pi[:], lhsT=ones_row[:], rhs=idx_row_f[:, qs],
                                 start=True, stop=True)
                for bh in range(2):
                    nc.vector.scalar_tensor_tensor(
                        G[:, bh, qs], pi[:], iota_f[bh][:, :1], r_bc[:],
                        op0=Alu.is_equal, op1=Alu.mult)
                pb = pspool.tile([P, 512], F32, tag="bc", bufs=2, name=f"pb{q}")
                nc.tensor.matmul(pb[:], lhsT=ones_row[:], rhs=bias_row[:, qs],
                                 start=True, stop=True)
                nc.scalar.copy(bias_bc[:, qs], pb[:])

            # ---- basisT [128 i, KC k, 256 b] bf16 --------------------
            basisT = cpool.tile([P, KC, BASIS], BF16, name="basisT")
            for bh2 in range(2):
                b_bf = xbfpool.tile([P, D_IN], BF16, tag="xbf",
                                    name=f"bbf{bh2}")
                nc.gpsimd.dma_start(b_bf[:], basis_in[bh2 * P:(bh2 + 1) * P, :])
                nc.sync.dma_start_transpose(
                    basisT[:, :, bh2 * P:(bh2 + 1) * P], b_bf[:])

            # ---- persistent W^T (residual, fp8) and zT ---------------
            residT = cpool.tile([P, KC, O_SH], F8, name="residT")
            zT = cpool.tile([P, 2, N_SH], BF16, name="zT")

            def resid_step(t, hf):
                # [128 o-rows, 2048 i-half]: load, decode, transpose, cast
                hs = slice(hf * HALF, (hf + 1) * HALF)
                eng_a = nc.sync if (2 * t + hf) % 2 == 0 else nc.scalar
                r32 = r32pool.tile([P, HALF], I32, tag="r32",
                                   name=f"r32_{t}_{hf}")
                eng_a.dma_start(r32[:], resid_in[t * P:(t + 1) * P, hs])
                rbf = rbfpool.tile([P, HALF], BF16, tag="rbf",
                                   name=f"rbf{t}_{hf}")
                nc.vector.tensor_scalar(rbf[:], r32[:], sv_pp[:, t:t + 1],
                                        bv_pp[:, t:t + 1], Alu.mult, Alu.add)
                # PE transpose (xbar is descriptor-bound); evac casts to f8
                for g in range(4):
                    prt = pspool.tile([P, 512], BF16, tag="bc", bufs=2,
                                      name=f"prt{t}_{hf}_{g}")
                    for j in range(4):
                        kk = 4 * g + j
                        nc.tensor.transpose(
                            prt[:, j * P:(j + 1) * P],
                            rbf[:, kk * P:(kk + 1) * P], identity[:])
                    k0 = hf * KH + 4 * g
                    nc.scalar.copy(
                        residT[:, k0:k0 + 4, t * P:(t + 1) * P], prt[:])

            def xz_block(tb):
                # one token block: x -> xT (bf16+f8), z chain, zT via PE
                xbf = xbfpool.tile([P, D_IN], BF16, tag="xbf",
                                   name=f"xbf{tb}")
                nc.gpsimd.dma_start(xbf[:], x_in[tb * P:(tb + 1) * P, :])
                xt = xtpool.tile([P, KC, P], BF16, tag="xt", name=f"xt{tb}")
                nc.sync.dma_start_transpose(xt[:], xbf[:])
                xf8 = xf8pool.tile([P, KC, P], F8, tag="xf8", name=f"xf8_{tb}")
                nc.vector.tensor_copy(xf8[:], xt[:])
                # z[tok, b] with full-width rhs, then PE-transpose to zT
                pz = pspool.tile([P, 512], F32, tag="z", bufs=2,
                                 name=f"pz{tb}")
                for k in range(KC):
                    nc.tensor.matmul(pz[:, :2 * P], lhsT=xt[:, k, :],
                                     rhs=basisT[:, k, :],
                                     start=(k == 0), stop=(k == KC - 1))
                z_sb = zsbpool.tile([P, 2 * P], BF16, tag="z", name=f"zsb{tb}")
                nc.scalar.copy(z_sb[:], pz[:, :2 * P])
                for bh in range(2):
                    # full-bank PSUM tile: sub-bank tiles corrupt each
                    # other through the bank-granular start=True zeroing
                    pzT = pspool.tile([P, 1024], BF16, tag="zt", bufs=1,
                                      name=f"pzT{tb}_{bh}")
                    nc.tensor.transpose(pzT[:, :P], z_sb[:, bh * P:(bh + 1) * P],
                                        identity[:])
                    nc.scalar.copy(zT[:, bh, tb * P:(tb + 1) * P], pzT[:, :P])
                return xf8

            xf8s = [None] * NB

            def main_mm(tb, os):
                osl = slice(os * 512, (os + 1) * 512)
                tbs = slice(tb * P, (tb + 1) * P)
                xf8 = xf8s[tb]
                ps = pspool.tile([P, 512], F32, tag="mm", bufs=3,
                                 name=f"ps{tb}_{os}")
                for bh in range(2):
                    nc.tensor.matmul(ps[:], lhsT=zT[:, bh, tbs],
                                     rhs=G[:, bh, osl],
                                     start=(bh == 0), stop=False)
                for k2 in range(KH):
                    ks = slice(2 * k2, 2 * k2 + 2)
                    nc.tensor.matmul(ps[:], lhsT=xf8[:, ks, :],
                                     rhs=residT[:, ks, osl],
                                     start=False, stop=(k2 == KH - 1),
                                     perf_mode=DR)
                y_t = ypool.tile([P, 512], F32, tag="y", name=f"y{tb}_{os}")
                nc.vector.scalar_tensor_tensor(y_t[:], ps[:], 1.0 / CSC,
                                               bias_bc[:, osl],
                                               op0=Alu.mult, op1=Alu.add)
                nc.scalar.dma_start(y_out[tbs, osl], y_t[:])

            # ---- interleaved schedule (issue order = engine order) ---
            for t in (0, 1, 2, 3):
                resid_step(t, 0), resid_step(t, 1)
            xf8s[0] = xz_block(0)
            xf8s[1] = xz_block(1)
            for t in (4, 5):
                resid_step(t, 0), resid_step(t, 1)
            xf8s[2] = xz_block(2)
            xf8s[3] = xz_block(3)
            for t in (6, 7):
                resid_step(t, 0), resid_step(t, 1)
            main_mm(0, 0), main_mm(1, 0)
            for t in (8, 9):
                resid_step(t, 0), resid_step(t, 1)
            main_mm(2, 0), main_mm(3, 0)
            for t in (10, 11):
                resid_step(t, 0), resid_step(t, 1)
            main_mm(0, 1), main_mm(1, 1)
            for t in (12, 13):
                resid_step(t, 0), resid_step(t, 1)
            main_mm(2, 1), main_mm(3, 1)
            for t in (14, 15):
                resid_step(t, 0), resid_step(t, 1)
            for tb in range(4):
                main_mm(tb, 2)
            for tb in range(4):
                main_mm(tb, 3)
            xf8s[4] = xz_block(4)
            for tb in range(4, NB):
                if tb + 1 < NB:
                    xf8s[tb + 1] = xz_block(tb + 1)
                for os in range(NOS):
                    main_mm(tb, os)

    if split_waits:
        _split_sync_waits(nc)
    return nc


_program_cache = {}


def _get_program():
    if "nc" not in _program_cache:
        _program_cache["nc"] = _build_program()
    return _program_cache["nc"]


def _in_maps(x, codes, basis_table, residual_q, residual_scales, bias):
    x2 = x.reshape(B * S, D_IN)
    in_maps = []
    for core in range(N_CORES):
        oc, nb = divmod(core, N_SHARDS)
        osl = slice(oc * O_SH, (oc + 1) * O_SH)
        nsl = slice(nb * N_SH, (nb + 1) * N_SH)
        in_maps.append({
            "x_sh": np.ascontiguousarray(x2[nsl]),
            "codes_sh": np.ascontiguousarray(codes[osl]),
            "basis": basis_table,
            "resid_sh": np.ascontiguousarray(residual_q[osl]),
            "scales_sh": np.ascontiguousarray(residual_scales[osl]),
            "bias_sh": np.ascontiguousarray(bias[osl]),
        })
    return in_maps


def kernel(x, codes, basis_table, residual_q, residual_scales, bias):
    x = np.ascontiguousarray(np.asarray(x, dtype=np.float32))
    codes = np.ascontiguousarray(np.asarray(codes, dtype=np.int32))
    basis_table = np.ascontiguousarray(np.asarray(basis_table, dtype=np.float32))
    residual_q = np.ascontiguousarray(np.asarray(residual_q, dtype=np.int32))
    residual_scales = np.ascontiguousarray(
        np.asarray(residual_scales, dtype=np.float32))
    bias = np.ascontiguousarray(np.asarray(bias, dtype=np.float32))

    nc = _get_program()
    res = run_bass_kernel_spmd(nc, _in_maps(x, codes, basis_table, residual_q,
                                            residual_scales, bias),
                               core_ids=list(range(N_CORES)))

    y = np.empty((B * S, D_OUT), dtype=np.float32)
    for core in range(N_CORES):
        oc, nb = divmod(core, N_SHARDS)
        y[nb * N_SH:(nb + 1) * N_SH, oc * O_SH:(oc + 1) * O_SH] = \
            res.results[core]["y_sh"]
    return y.reshape(B, S, D_OUT)


# revision 16
# speedup vs baseline: 1.5056x; 1.0526x over previous
"""BitfieldLinear (vq_codebook) Trainium2 kernel — fp8 decomposed.

y = x @ W^T + bias, W = r[:,None]*basis[idx] + s[:,None]*(q-128)/127.

Instead of materializing W in bf16 (PE floor ~437us/core), split:
  y = zT.T @ G + x_f8 @ residT_f8 * (1/C) + bias
  - z = x @ basisT in bf16 ([tokens, 256 basis]); the basis term
    becomes a one-hot gather matmul with G[b,o] = C*r[o] at b=idx[o]
    (accurate path, 1/8 the flops).
  - the residual matmul runs in fp8e4 DoubleRow perf mode (2x PE rate);
    residT_f8 pre-scaled by s*C/127 (C=2048 keeps values out of the
    fp8 subnormal range), x cast bf16->fp8 directly.
  Both accumulate into the same PSUM bank; evacuation is one DVE op
  (psum * 1/C + bias_bc).

Sharding across 8 NeuronCores: 2-way over out_features x 4-way over
flattened tokens. Per core: 2048 tokens x 2048 outs, K=4096.
  - resid rows stream: int32 DMA -> ACT/DVE decode (s*C/127 folded,
    bf16) -> xbar transpose -> DVE cast to residT_f8 [128,32k,2048o].
  - x streams via gpsimd cast-DMA (f32->bf16) -> xbar -> fp8 cast;
    z chains + PE transposes of z run during the resid build window.
Host only slices inputs and reassembles the output.
"""

import numpy as np

import concourse.bass as bass
import concourse.mybir as mybir
import concourse.tile as tile
from concourse.masks import make_identity
from concourse.bass_utils import run_bass_kernel_spmd

# problem shape (hardcoded per harness contract)
B, S, D_IN, D_OUT, BASIS = 4, 2048, 4096, 4096, 256
N_CORES = 8
O_SHARDS, N_SHARDS = 2, 4           # grid: core = oc * N_SHARDS + nb
O_SH = D_OUT // O_SHARDS            # 2048 out-features per core
N_SH = (B * S) // N_SHARDS          # 2048 token rows per core

P = 128
KC = D_IN // P                      # 32 contraction chunks
KH = KC // 2                        # 16 chunks per D_IN half
HALF = D_IN // 2
NB = N_SH // P                      # 16 token blocks per core
NOS = O_SH // 512                   # 4 PSUM o-slices per core
OT = O_SH // P                      # 16 resid row-tiles per core
CSC = 2048.0                        # fp8 residual pre-scale

F32 = mybir.dt.float32
BF16 = mybir.dt.bfloat16
F8 = mybir.dt.float8e4
I32 = mybir.dt.int32

_WAIT_LIMIT = 1


def _split_sync_waits(nc):
    """walrus in this container rejects instructions with more than one
    embedded sync-wait command; hoist the excess onto same-engine NoOps."""
    ctr = 0
    for f in nc.m.functions:
        for bb in f.blocks:
            new = []
            changed = False
            for inst in bb.instructions:
                si = inst.sync_info
                if si is not None and si.on_wait and len(si.on_wait) > _WAIT_LIMIT:
                    waits = list(si.on_wait)
                    excess, keep = waits[:-_WAIT_LIMIT], waits[-_WAIT_LIMIT:]
                    for i in range(0, len(excess), _WAIT_LIMIT):
                        ctr += 1
                        new.append(mybir.InstNoOp(
                            name=f"I-waitsplit-{ctr}",
                            engine=inst.engine,
                            ins=[], outs=[],
                            sync_info=mybir.SyncInfo(
                                on_wait=excess[i:i + _WAIT_LIMIT], on_update=[]),
                        ))
                    si.on_wait = keep
                    changed = True
                new.append(inst)
            if changed:
                bb.instructions = new


def _build_program(split_waits=True):
    nc = bass.Bass()
    Alu = mybir.AluOpType
    Act = mybir.ActivationFunctionType
    DR = mybir.MatmulPerfMode.DoubleRow

    x_in = nc.dram_tensor("x_sh", [N_SH, D_IN], F32, kind="ExternalInput")
    codes_in = nc.dram_tensor("codes_sh", [O_SH], I32, kind="ExternalInput")
    basis_in = nc.dram_tensor("basis", [BASIS, D_IN], F32, kind="ExternalInput")
    resid_in = nc.dram_tensor("resid_sh", [O_SH, D_IN], I32, kind="ExternalInput")
    scales_in = nc.dram_tensor("scales_sh", [O_SH], F32, kind="ExternalInput")
    bias_in = nc.dram_tensor("bias_sh", [O_SH], F32, kind="ExternalInput")
    y_out = nc.dram_tensor("y_sh", [N_SH, O_SH], F32, kind="ExternalOutput")

    with tile.TileContext(nc) as tc:
        with (
            tc.tile_pool(name="const", bufs=1) as cpool,
            tc.tile_pool(name="rows", bufs=2) as rowpool,   # i32 scratch rows
            tc.tile_pool(name="xbf", bufs=2) as xbfpool,    # [128,4096] bf16
            tc.tile_pool(name="xt", bufs=2) as xtpool,      # [128,KC,128] bf16
            tc.tile_pool(name="xf8", bufs=4) as xf8pool,    # [128,KC,128] f8
            tc.tile_pool(name="zsb", bufs=2) as zsbpool,    # [128,256] bf16
            tc.tile_pool(name="r32", bufs=2) as r32pool,    # [128,2048] i32
            tc.tile_pool(name="rbf", bufs=2) as rbfpool,    # [128,HALF] bf16
            tc.tile_pool(name="y", bufs=2) as ypool,        # [128,512] f32
            tc.tile_pool(name="psum", bufs=6, space="PSUM") as pspool,
        ):
            # ---- decode code scalars --------------------------------
            codes_row = rowpool.tile([1, O_SH], I32, tag="row",
                                     name="codes_row")
            nc.sync.dma_start(codes_row[:], codes_in[None, :])
            tmp_row = rowpool.tile([1, O_SH], I32, tag="row", name="tmp_row")
            nc.vector.tensor_scalar(tmp_row[:], codes_row[:], 0xFF, None,
                                    Alu.bitwise_and)
            idx_row_f = cpool.tile([1, O_SH], BF16, name="idx_row_f")
            nc.scalar.activation(idx_row_f[:], tmp_row[:], Act.Copy)
            rq_row = rowpool.tile([1, O_SH], I32, tag="row", name="rq_row")
            nc.vector.tensor_scalar(rq_row[:], codes_row[:], 8, 0xFFFF,
                                    Alu.logical_shift_right, Alu.bitwise_and)
            # r scaled by C so the fp8 residual path's 1/C evac matches
            r_row_f = cpool.tile([1, O_SH], BF16, name="r_row_f")
            nc.scalar.activation(r_row_f[:], rq_row[:], Act.Copy,
                                 scale=CSC / 65535.0)

            # per-row decode scale/bias for the residual (ACT layout)
            s_pp = cpool.tile([P, OT], F32, name="s_pp")
            nc.sync.dma_start(s_pp[:], scales_in.rearrange("(t p) -> p t", p=P))
            sv_pp = cpool.tile([P, OT], F32, name="sv_pp")
            nc.vector.tensor_scalar_mul(sv_pp[:], s_pp[:], CSC / 127.0)
            bv_pp = cpool.tile([P, OT], F32, name="bv_pp")
            nc.vector.tensor_scalar_mul(bv_pp[:], s_pp[:], -128.0 * CSC / 127.0)

            bias_row = cpool.tile([1, O_SH], BF16, name="bias_row")
            nc.gpsimd.dma_start(bias_row[:], bias_in[None, :])
            ones_row = cpool.tile([1, P], BF16, name="ones_row")
            nc.vector.memset(ones_row[:], 1.0)
            identity = cpool.tile([P, P], BF16, name="identity")
            make_identity(nc, identity[:])

            # ---- one-hot G [128 b, 2 bh, O_SH o] and bias broadcast --
            iota_i = cpool.tile([P, 1], I32, name="iota_i")
            nc.gpsimd.iota(iota_i[:], pattern=[[0, 1]], base=0,
                           channel_multiplier=1)
            iota_f = [cpool.tile([P, 1], F32, name=f"iota_f{bh}")
                      for bh in range(2)]
            nc.scalar.activation(iota_f[0][:], iota_i[:], Act.Copy)
            nc.scalar.activation(iota_f[1][:], iota_i[:], Act.Copy, bias=128.0,
                                 scale=1.0)
            G = cpool.tile([P, 2, O_SH], BF16, name="G")
            bias_bc = cpool.tile([P, O_SH], BF16, name="bias_bc")
            r_bc = cpool.tile([P, 512], BF16, name="r_bc")
            for q in range(NOS):
                qs = slice(q * 512, (q + 1) * 512)
                pr = pspool.tile([P, 512], F32, tag="bc", bufs=2, name=f"pr{q}")
                nc.tensor.matmul(pr[:], lhsT=ones_row[:], rhs=r_row_f[:, qs],
                                 start=True, stop=True)
                nc.scalar.copy(r_bc[:], pr[:])
                pi = pspool.tile([P, 512], F32, tag="bc", bufs=2, name=f"pi{q}")
                nc.tensor.matmul(pi[:], lhsT=ones_row[:], rhs=idx_row_f[:, qs],
                                 start=True, stop=True)
                for bh in range(2):
                    nc.vector.scalar_tensor_tensor(
                        G[:, bh, qs], pi[:], iota_f[bh][:, :1], r_bc[:],
                        op0=Alu.is_equal, op1=Alu.mult)
                pb = pspool.tile([P, 512], F32, tag="bc", bufs=2, name=f"pb{q}")
                nc.tensor.matmul(pb[:], lhsT=ones_row[:], rhs=bias_row[:, qs],
                                 start=True, stop=True)
                nc.scalar.copy(bias_bc[:, qs], pb[:])

            # ---- basisT [128 i, KC k, 256 b] bf16 --------------------
            basisT = cpool.tile([P, KC, BASIS], BF16, name="basisT")
            for bh2 in range(2):
                b_bf = xbfpool.tile([P, D_IN], BF16, tag="xbf",
                                    name=f"bbf{bh2}")
                nc.gpsimd.dma_start(b_bf[:], basis_in[bh2 * P:(bh2 + 1) * P, :])
                nc.sync.dma_start_transpose(
                    basisT[:, :, bh2 * P:(bh2 + 1) * P], b_bf[:])

            # ---- persistent W^T (residual, fp8) and zT ---------------
            residT = cpool.tile([P, KC, O_SH], F8, name="residT")
            zT = cpool.tile([P, 2, N_SH], BF16, name="zT")

            def resid_step(t, hf):
                # [128 o-rows, 2048 i-half]: load, decode, transpose, cast
                hs = slice(hf * HALF, (hf + 1) * HALF)
                eng_a = nc.sync if (2 * t + hf) % 2 == 0 else nc.scalar
                r32 = r32pool.tile([P, HALF], I32, tag="r32",
                                   name=f"r32_{t}_{hf}")
                eng_a.dma_start(r32[:], resid_in[t * P:(t + 1) * P, hs])
                rbf = rbfpool.tile([P, HALF], BF16, tag="rbf",
                                   name=f"rbf{t}_{hf}")
                nc.scalar.activation(rbf[:], r32[:], Act.Identity,
                                     bias=bv_pp[:, t:t + 1],
                                     scale=sv_pp[:, t:t + 1])
                # PE transpose (xbar is descriptor-bound); evac casts to f8
                for g in range(4):
                    prt = pspool.tile([P, 512], BF16, tag="bc", bufs=2,
                                      name=f"prt{t}_{hf}_{g}")
                    for j in range(4):
                        kk = 4 * g + j
                        nc.tensor.transpose(
                            prt[:, j * P:(j + 1) * P],
                            rbf[:, kk * P:(kk + 1) * P], identity[:])
                    k0 = hf * KH + 4 * g
                    nc.vector.tensor_copy(
                        residT[:, k0:k0 + 4, t * P:(t + 1) * P], prt[:])

            def x_load(tb):
                xbf = xbfpool.tile([P, D_IN], BF16, tag="xbf",
                                   name=f"xbf{tb}")
                nc.gpsimd.dma_start(xbf[:], x_in[tb * P:(tb + 1) * P, :])
                return xbf

            def xz_block(tb, xbf):
                # one token block: xT (bf16+f8), z chain, zT via PE
                xt = xtpool.tile([P, KC, P], BF16, tag="xt", name=f"xt{tb}")
                nc.sync.dma_start_transpose(xt[:], xbf[:])
                xf8 = xf8pool.tile([P, KC, P], F8, tag="xf8", name=f"xf8_{tb}")
                nc.vector.tensor_copy(xf8[:], xt[:])
                # z[tok, b] with full-width rhs, then PE-transpose to zT
                pz = pspool.tile([P, 512], F32, tag="z", bufs=2,
                                 name=f"pz{tb}")
                for k in range(KC):
                    nc.tensor.matmul(pz[:, :2 * P], lhsT=xt[:, k, :],
                                     rhs=basisT[:, k, :],
                                     start=(k == 0), stop=(k == KC - 1))
                z_sb = zsbpool.tile([P, 2 * P], BF16, tag="z", name=f"zsb{tb}")
                nc.scalar.copy(z_sb[:], pz[:, :2 * P])
                for bh in range(2):
                    # full-bank PSUM tile: sub-bank tiles corrupt each
                    # other through the bank-granular start=True zeroing
                    pzT = pspool.tile([P, 1024], BF16, tag="zt", bufs=1,
                                      name=f"pzT{tb}_{bh}")
                    nc.tensor.transpose(pzT[:, :P], z_sb[:, bh * P:(bh + 1) * P],
                                        identity[:])
                    nc.scalar.copy(zT[:, bh, tb * P:(tb + 1) * P], pzT[:, :P])
                return xf8

            xf8s = [None] * NB

            def main_mm(tb, os):
                osl = slice(os * 512, (os + 1) * 512)
                tbs = slice(tb * P, (tb + 1) * P)
                xf8 = xf8s[tb]
                ps = pspool.tile([P, 512], F32, tag="mm", bufs=3,
                                 name=f"ps{tb}_{os}")
                for bh in range(2):
                    nc.tensor.matmul(ps[:], lhsT=zT[:, bh, tbs],
                                     rhs=G[:, bh, osl],
                                     start=(bh == 0), stop=False)
                for k2 in range(KH):
                    ks = slice(2 * k2, 2 * k2 + 2)
                    nc.tensor.matmul(ps[:], lhsT=xf8[:, ks, :],
                                     rhs=residT[:, ks, osl],
                                     start=False, stop=(k2 == KH - 1),
                                     perf_mode=DR)
                y_t = ypool.tile([P, 512], F32, tag="y", name=f"y{tb}_{os}")
                nc.vector.scalar_tensor_tensor(y_t[:], ps[:], 1.0 / CSC,
                                               bias_bc[:, osl],
                                               op0=Alu.mult, op1=Alu.add)
                nc.scalar.dma_start(y_out[tbs, osl], y_t[:])

            # ---- interleaved schedule (issue order = engine order) ---
            x_loads = [None] * NB
            x_loads[0] = x_load(0)
            x_loads[1] = x_load(1)
            for t in (0, 1, 2, 3):
                resid_step(t, 0), resid_step(t, 1)
            xf8s[0] = xz_block(0, x_loads[0])
            x_loads[2] = x_load(2)
            xf8s[1] = xz_block(1, x_loads[1])
            x_loads[3] = x_load(3)
            for t in (4, 5):
                resid_step(t, 0), resid_step(t, 1)
            xf8s[2] = xz_block(2, x_loads[2])
            x_loads[4] = x_load(4)
            xf8s[3] = xz_block(3, x_loads[3])
            x_loads[5] = x_load(5)
            for t in (6, 7):
                resid_step(t, 0), resid_step(t, 1)
            main_mm(0, 0), main_mm(1, 0)
            for t in (8, 9):
                resid_step(t, 0), resid_step(t, 1)
            main_mm(2, 0), main_mm(3, 0)
            for t in (10, 11):
                resid_step(t, 0), resid_step(t, 1)
            main_mm(0, 1), main_mm(1, 1)
            for t in (12, 13):
                resid_step(t, 0), resid_step(t, 1)
            main_mm(2, 1), main_mm(3, 1)
            for t in (14, 15):
                resid_step(t, 0), resid_step(t, 1)
            for tb in range(4):
                main_mm(tb, 2)
            for tb in range(4):
                main_mm(tb, 3)
            xf8s[4] = xz_block(4, x_loads[4])
            x_loads[6] = x_load(6)
            for tb in range(4, NB):
                if tb + 1 < NB:
                    xf8s[tb + 1] = xz_block(tb + 1, x_loads[tb + 1])
                    if tb + 3 < NB:
                        x_loads[tb + 3] = x_load(tb + 3)
                for os in range(NOS):
                    main_mm(tb, os)

    if split_waits:
        _split_sync_waits(nc)
    return nc


_program_cache = {}


def _get_program():
    if "nc" not in _program_cache:
        _program_cache["nc"] = _build_program()
    return _program_cache["nc"]


def _in_maps(x, codes, basis_table, residual_q, residual_scales, bias):
    x2 = x.reshape(B * S, D_IN)
    in_maps = []
    for core in range(N_CORES):
        oc, nb = divmod(core, N_SHARDS)
        osl = slice(oc * O_SH, (oc + 1) * O_SH)
        nsl = slice(nb * N_SH, (nb + 1) * N_SH)
        in_maps.append({
            "x_sh": np.ascontiguousarray(x2[nsl]),
            "codes_sh": np.ascontiguousarray(codes[osl]),
            "basis": basis_table,
            "resid_sh": np.ascontiguousarray(residual_q[osl]),
            "scales_sh": np.ascontiguousarray(residual_scales[osl]),
            "bias_sh": np.ascontiguousarray(bias[osl]),
        })
    return in_maps


def kernel(x, codes, basis_table, residual_q, residual_scales, bias):
    x = np.ascontiguousarray(np.asarray(x, dtype=np.float32))
    codes = np.ascontiguousarray(np.asarray(codes, dtype=np.int32))
    basis_table = np.ascontiguousarray(np.asarray(basis_table, dtype=np.float32))
    residual_q = np.ascontiguousarray(np.asarray(residual_q, dtype=np.int32))
    residual_scales = np.ascontiguousarray(
        np.asarray(residual_scales, dtype=np.float32))
    bias = np.ascontiguousarray(np.asarray(bias, dtype=np.float32))

    nc = _get_program()
    res = run_bass_kernel_spmd(nc, _in_maps(x, codes, basis_table, residual_q,
                                            residual_scales, bias),
                               core_ids=list(range(N_CORES)))

    y = np.empty((B * S, D_OUT), dtype=np.float32)
    for core in range(N_CORES):
        oc, nb = divmod(core, N_SHARDS)
        y[nb * N_SH:(nb + 1) * N_SH, oc * O_SH:(oc + 1) * O_SH] = \
            res.results[core]["y_sh"]
    return y.reshape(B, S, D_OUT)
